# revision 1
# baseline (speedup 1.0000x reference)
"""TAGConv×2 GNN discriminator on 8 Trainium2 NeuronCores.

Strategy (per sharding hint): shard destination nodes across 8 cores
(6272 padded rows each), replicate per-hop weights, exchange the
propagated node-feature table between hops via AllGather, all-reduce the
global pool result.

Device algorithm per A-application ("hop"):
  - dma_gather pulls 128-edge chunks of source-node feature rows (bf16,
    256B rows) from the replicated DRAM table (split in two halves to fit
    int16 gather indices).
  - A selection matrix S[e, r] = norm[e] * (rowloc[e] == r), built on the
    vector engine from compact per-edge metadata, scatter-adds each chunk
    into a PSUM block via a TensorE matmul (out[128 rows, 128 feat] +=
    S^T @ gathered).
  - The dense Horner term x@W[k] (or h1@W2[k]) accumulates into the same
    PSUM tile, so out = x@W[k] + A·t comes out of PSUM directly.
Layer ends apply bias + PReLU; layer 2's final hop feeds a masked pool
matmul; the pooled vector hits Wout and an AllReduce finishes the job.
"""
import hashlib
import math
import os

import numpy as np
import ml_dtypes

BF = ml_dtypes.bfloat16

NCORES = 8
NNODES = 50000
FEAT = 128
KHOPS = 3
NBLK = 49                 # 128-row blocks per core
RPC = NBLK * 128          # rows per core (6272)
NPAD = NCORES * RPC       # padded node count (50176)
SPLIT = 32768             # int16 gather-index split point
GIDX = 1024               # indices per dma_gather call
SGRP = 32                 # subchunks per S-build DVE op

_CACHE: dict = {}

# dma_gather cannot read internal (scratchpad-relative) DRAM — tables must be
# I/O tensors. If True, AllGather writes the ExternalOutput table directly;
# if False, AllGather lands in an internal Shared bounce and a dram2dram copy
# moves it into the ExternalOutput table.
DIRECT_CC_TABLES = False
# Debug: emit only the first K_STEPS stages (1..8); 8 = full program.
K_STEPS = int(os.environ.get("K_STEPS", "8"))


# ----------------------------------------------------------------------------
# Host preprocessing: graph -> per-core gather streams + program structure
# ----------------------------------------------------------------------------

def _preprocess(edge_index: np.ndarray):
    row = edge_index[0].astype(np.int64)
    col = edge_index[1].astype(np.int64)
    E = row.shape[0]

    deg = np.bincount(row, minlength=NNODES).astype(np.float64)
    dinv = np.where(deg > 0, 1.0 / np.sqrt(np.maximum(deg, 1.0)), 0.0)
    norm = (dinv[row] * dinv[col]).astype(np.float32)

    gb = row >> 7                       # global 128-row block (0..391)
    half = (col >= SPLIT).astype(np.int64)
    order = np.lexsort((half, gb))
    gb_s, half_s, col_s, row_s, norm_s = (
        gb[order], half[order], col[order], row[order], norm[order])

    # counts per (global block, half)
    key = gb_s * 2 + half_s
    cnt = np.bincount(key, minlength=NCORES * NBLK * 2)
    cnt_lo = cnt[0::2].reshape(NCORES, NBLK)
    cnt_hi = cnt[1::2].reshape(NCORES, NBLK)
    CLO = np.maximum(1, np.ceil(cnt_lo.max(0) / 128).astype(np.int64))
    CHI = np.maximum(1, np.ceil(cnt_hi.max(0) / 128).astype(np.int64))

    LLO = int(CLO.sum()) * 128
    LHI = int(CHI.sum()) * 128
    CTOT = int(CLO.sum() + CHI.sum())

    starts = np.zeros(NCORES * NBLK * 2 + 1, np.int64)
    np.cumsum(cnt, out=starts[1:])

    per_core = []
    for c in range(NCORES):
        idx_lo = np.zeros(LLO, np.int16)
        idx_hi = np.zeros(LHI, np.int16)
        rowloc = np.zeros(CTOT * 128, np.float32)
        normv = np.zeros(CTOT * 128, np.float32)
        plo = phi = ps = 0
        for b in range(NBLK):
            k2 = (c * NBLK + b) * 2
            s0, s1 = starts[k2], starts[k2 + 1]
            nlo = s1 - s0
            idx_lo[plo:plo + nlo] = col_s[s0:s1].astype(np.int16)
            rowloc[ps:ps + nlo] = (row_s[s0:s1] & 127).astype(np.float32)
            normv[ps:ps + nlo] = norm_s[s0:s1]
            plo += CLO[b] * 128
            ps += CLO[b] * 128
            s0, s1 = starts[k2 + 1], starts[k2 + 2]
            nhi = s1 - s0
            idx_hi[phi:phi + nhi] = (col_s[s0:s1] - SPLIT).astype(np.int16)
            rowloc[ps:ps + nhi] = (row_s[s0:s1] & 127).astype(np.float32)
            normv[ps:ps + nhi] = norm_s[s0:s1]
            phi += CHI[b] * 128
            ps += CHI[b] * 128

        idx_lo_w = np.tile(idx_lo.reshape(-1, 16).T, (8, 1)).astype(np.int16)
        idx_hi_w = np.tile(idx_hi.reshape(-1, 16).T, (8, 1)).astype(np.int16)
        rowloc_w = np.ascontiguousarray(rowloc.reshape(CTOT, 128).T).astype(BF)
        normv_w = np.ascontiguousarray(normv.reshape(CTOT, 128).T).astype(BF)
        per_core.append(dict(idx_lo=idx_lo_w, idx_hi=idx_hi_w,
                             rowloc=rowloc_w, normv=normv_w))

    struct = dict(CLO=tuple(int(v) for v in CLO), CHI=tuple(int(v) for v in CHI),
                  LLO=LLO, LHI=LHI, CTOT=CTOT)
    return struct, per_core


# ----------------------------------------------------------------------------
# Bass program
# ----------------------------------------------------------------------------

def _build_program(struct):
    import concourse.bacc as bacc
    import concourse.mybir as mybir
    import concourse.tile as tile

    CLO, CHI = struct["CLO"], struct["CHI"]
    LLO, LHI, CTOT = struct["LLO"], struct["LHI"], struct["CTOT"]
    NLO_CALLS = math.ceil(LLO / GIDX)
    NHI_CALLS = math.ceil(LHI / GIDX)
    NSGRP = math.ceil(CTOT / SGRP)
    f32 = mybir.dt.float32
    bf16 = mybir.dt.bfloat16
    i16 = mybir.dt.int16

    nc = bacc.Bacc("TRN2", target_bir_lowering=False, debug=False,
                   num_devices=NCORES, dynamic_dma_scratch_size=32768)

    P = {}
    def param(name, shape, dt):
        P[name] = nc.declare_dram_parameter(name, list(shape), dt, isOutput=False)
        return P[name]

    param("idx_lo", [128, LLO // 16], i16)
    param("idx_hi", [128, LHI // 16], i16)
    param("rowloc", [128, CTOT], bf16)
    param("normv", [128, CTOT], bf16)
    param("xT", [128, RPC], bf16)
    param("w1", [128, KHOPS + 1, 128], bf16)
    param("w2", [128, KHOPS + 1, 128], bf16)
    param("b1f", [128, 128], f32)
    param("b2f", [128, 128], f32)
    param("a1c", [128, 1], f32)
    param("a2c", [128, 1], f32)
    param("wout", [128, 1], f32)
    param("boutc", [1, 1], f32)
    param("maskc", [128, NBLK], bf16)
    param("iotam", [128, 128], bf16)
    param("ident", [128, 128], f32)
    out_ext = nc.declare_dram_parameter("out", [1, 1], f32, isOutput=True)
    tablesA = [nc.declare_dram_parameter(f"tabA{i}", [SPLIT, FEAT], bf16,
                                         isOutput=True) for i in range(6)]
    tablesB = [nc.declare_dram_parameter(f"tabB{i}", [NPAD - SPLIT, FEAT], bf16,
                                         isOutput=True) for i in range(6)]

    RG = [list(range(NCORES))]

    with tile.TileContext(nc) as tc:
        with (
            tc.tile_pool(name="const", bufs=1) as cpool,
            tc.tile_pool(name="shardp", bufs=2) as shpool,
            tc.tile_pool(name="glo", bufs=8) as glop,
            tc.tile_pool(name="ghi", bufs=8) as ghip,
            tc.tile_pool(name="sgrp", bufs=4) as sgp,
            tc.tile_pool(name="work", bufs=3) as wkp,
            tc.tile_pool(name="ps", bufs=4, space="PSUM") as psp,
            tc.tile_pool(name="psx", bufs=1, space="PSUM") as psx,
            tc.tile_pool(name="dram", bufs=1, space="DRAM") as drp,
        ):
            # ---- resident constants ----
            def cload(name, shape, dt, tag):
                t = cpool.tile(shape, dt, tag=tag)
                nc.sync.dma_start(out=t[:], in_=P[name][:])
                return t

            idxlo_t = cload("idx_lo", [128, LLO // 16], i16, "idxlo")
            idxhi_t = cload("idx_hi", [128, LHI // 16], i16, "idxhi")
            rowloc_t = cload("rowloc", [128, CTOT], bf16, "rowloc")
            normv_t = cload("normv", [128, CTOT], bf16, "normv")
            xT_t = cload("xT", [128, RPC], bf16, "xT")
            w1_t = cload("w1", [128, KHOPS + 1, 128], bf16, "w1")
            w2_t = cload("w2", [128, KHOPS + 1, 128], bf16, "w2")
            b1f_t = cload("b1f", [128, 128], f32, "b1f")
            b2f_t = cload("b2f", [128, 128], f32, "b2f")
            a1c_t = cload("a1c", [128, 1], f32, "a1c")
            a2c_t = cload("a2c", [128, 1], f32, "a2c")
            wout_t = cload("wout", [128, 1], f32, "wout")
            bout_t = cload("boutc", [1, 1], f32, "bout")
            maskc_t = cload("maskc", [128, NBLK], bf16, "maskc")
            iota_t = cload("iotam", [128, 128], bf16, "iota")
            ident_t = cload("ident", [128, 128], f32, "ident")
            h1T_t = cpool.tile([128, RPC], bf16, tag="h1T")

            # ---- DRAM internals ----
            agbufs = [drp.tile([NPAD, FEAT], bf16, tag=f"agbuf{i}",
                               name=f"agbuf{i}", addr_space="Shared")
                      for i in range(6)]
            red_in = drp.tile([1, 1], f32, tag="red_in")
            red_out = drp.tile([1, 1], f32, tag="red_out", addr_space="Shared")

            def emit_gathers(ti):
                lo_bufs, hi_bufs = [], []
                for call in range(NLO_CALLS):
                    n = min(GIDX, LLO - call * GIDX)
                    gt = glop.tile([128, GIDX // 128, FEAT], bf16, tag="glo")
                    nc.gpsimd.dma_gather(
                        out_ap=gt[:, : n // 128, :],
                        in_ap=tablesA[ti][:],
                        idxs_ap=idxlo_t[:, call * (GIDX // 16):
                                        call * (GIDX // 16) + n // 16],
                        num_idxs=n, num_idxs_reg=n, elem_size=FEAT)
                    lo_bufs.append(gt)
                for call in range(NHI_CALLS):
                    n = min(GIDX, LHI - call * GIDX)
                    gt = ghip.tile([128, GIDX // 128, FEAT], bf16, tag="ghi")
                    nc.gpsimd.dma_gather(
                        out_ap=gt[:, : n // 128, :],
                        in_ap=tablesB[ti][:],
                        idxs_ap=idxhi_t[:, call * (GIDX // 16):
                                        call * (GIDX // 16) + n // 16],
                        num_idxs=n, num_idxs_reg=n, elem_size=FEAT)
                    hi_bufs.append(gt)
                return lo_bufs, hi_bufs

            def emit_sbuild():
                s_bufs = []
                for grp in range(NSGRP):
                    c0 = grp * SGRP
                    cn = min(SGRP, CTOT - c0)
                    st = sgp.tile([128, SGRP, 128], bf16, tag="sgrp")
                    nc.vector.tensor_tensor(
                        out=st[:, :cn, :],
                        in0=rowloc_t[:, c0:c0 + cn].unsqueeze(2)
                            .to_broadcast([128, cn, 128]),
                        in1=iota_t[:].unsqueeze(1).to_broadcast([128, cn, 128]),
                        op=mybir.AluOpType.is_equal)
                    nc.vector.tensor_tensor(
                        out=st[:, :cn, :],
                        in0=st[:, :cn, :],
                        in1=normv_t[:, c0:c0 + cn].unsqueeze(2)
                            .to_broadcast([128, cn, 128]),
                        op=mybir.AluOpType.mult)
                    s_bufs.append(st)
                return s_bufs

            def hop_blocks(ti, w_tile, k, lhsT_tile):
                """Yield (b, psum_tile) with the accumulated block result."""
                lo_bufs, hi_bufs = emit_gathers(ti)
                s_bufs = emit_sbuild()
                lo_sub = hi_sub = s_pos = 0
                for b in range(NBLK):
                    pt = psp.tile([128, FEAT], f32, tag="blk", space="PSUM")
                    first = True
                    for j in range(CLO[b]):
                        gs, si = lo_sub + j, s_pos + j
                        nc.tensor.matmul(
                            out=pt[:],
                            lhsT=s_bufs[si // SGRP][:, si % SGRP, :],
                            rhs=lo_bufs[gs // (GIDX // 128)][:, gs % (GIDX // 128), :],
                            start=first, stop=False)
                        first = False
                    s_pos += CLO[b]
                    for j in range(CHI[b]):
                        gs, si = hi_sub + j, s_pos + j
                        nc.tensor.matmul(
                            out=pt[:],
                            lhsT=s_bufs[si // SGRP][:, si % SGRP, :],
                            rhs=hi_bufs[gs // (GIDX // 128)][:, gs % (GIDX // 128), :],
                            start=first, stop=False)
                        first = False
                    s_pos += CHI[b]
                    lo_sub += CLO[b]
                    hi_sub += CHI[b]
                    nc.tensor.matmul(
                        out=pt[:], lhsT=lhsT_tile[:, 128 * b:128 * (b + 1)],
                        rhs=w_tile[:, k, :], start=first, stop=True)
                    yield b, pt

            def dense_blocks(w_tile, k, lhsT_tile):
                for b in range(NBLK):
                    pt = psp.tile([128, FEAT], f32, tag="blk", space="PSUM")
                    nc.tensor.matmul(
                        out=pt[:], lhsT=lhsT_tile[:, 128 * b:128 * (b + 1)],
                        rhs=w_tile[:, k, :], start=True, stop=True)
                    yield b, pt

            def store_and_gather_table(block_iter, ti):
                shard = shpool.tile([128, NBLK, FEAT], bf16, tag="shard")
                for b, pt in block_iter:
                    nc.any.tensor_copy(out=shard[:, b, :], in_=pt[:])
                bounce = drp.tile([RPC, FEAT], bf16, tag=f"bounce{ti}",
                                  name=f"bounce{ti}")
                nc.sync.dma_start(out=bounce[:], in_=shard[:])
                nc.gpsimd.collective_compute(
                    "AllGather", mybir.AluOpType.bypass, replica_groups=RG,
                    ins=[bounce.opt()], outs=[agbufs[ti].opt()])
                nc.sync.dma_start(out=tablesA[ti][:],
                                  in_=agbufs[ti][0:SPLIT, :])
                nc.sync.dma_start(out=tablesB[ti][:],
                                  in_=agbufs[ti][SPLIT:NPAD, :])

            def prelu(pt, bf_t, ac_t, dst_ap):
                biased = wkp.tile([128, 128], f32, tag="tmp1")
                neg = wkp.tile([128, 128], f32, tag="tmp2")
                nc.vector.tensor_tensor(out=biased[:], in0=pt[:], in1=bf_t[:],
                                        op=mybir.AluOpType.add)
                nc.vector.tensor_scalar(out=neg[:], in0=biased[:], scalar1=0.0,
                                        scalar2=ac_t[:, 0:1],
                                        op0=mybir.AluOpType.min,
                                        op1=mybir.AluOpType.mult)
                nc.vector.tensor_scalar(out=biased[:], in0=biased[:],
                                        scalar1=0.0, scalar2=None,
                                        op0=mybir.AluOpType.max)
                nc.vector.tensor_tensor(out=dst_ap, in0=biased[:], in1=neg[:],
                                        op=mybir.AluOpType.add)

            # ---- Layer 1, Horner ----
            # step 0: t = x @ W1[3]  -> T0
            store_and_gather_table(dense_blocks(w1_t, 3, xT_t), 0)
            # step 1: t = x @ W1[2] + A t -> T1
            if K_STEPS > 1:
                store_and_gather_table(hop_blocks(0, w1_t, 2, xT_t), 1)
            # step 2: t = x @ W1[1] + A t -> T2
            if K_STEPS > 2:
                store_and_gather_table(hop_blocks(1, w1_t, 1, xT_t), 2)
            # step 3: h1 = prelu(x @ W1[0] + A t + b1); keep h1T on chip
            if K_STEPS > 3:
                for b, pt in hop_blocks(2, w1_t, 0, xT_t):
                    h1f = wkp.tile([128, 128], f32, tag="h1f")
                    prelu(pt, b1f_t, a1c_t, h1f[:])
                    ptr = psx.tile([128, 128], f32, tag="tr", space="PSUM")
                    nc.tensor.transpose(out=ptr[:], in_=h1f[:], identity=ident_t[:])
                    nc.any.tensor_copy(out=h1T_t[:, 128 * b:128 * (b + 1)],
                                       in_=ptr[:])

            # ---- Layer 2, Horner ----
            # step 3.5: u = h1 @ W2[3] -> T3
            if K_STEPS > 4:
                store_and_gather_table(dense_blocks(w2_t, 3, h1T_t), 3)
            # step 4: u = h1 @ W2[2] + A u -> T4
            if K_STEPS > 5:
                store_and_gather_table(hop_blocks(3, w2_t, 2, h1T_t), 4)
            # step 5: u = h1 @ W2[1] + A u -> T5
            if K_STEPS > 6:
                store_and_gather_table(hop_blocks(4, w2_t, 1, h1T_t), 5)
            # step 6: h2 = prelu(h1 @ W2[0] + A u + b2); pool
            if K_STEPS > 7:
                pool_ps = psx.tile([128, 1], f32, tag="pool", space="PSUM")
                for b, pt in hop_blocks(5, w2_t, 0, h1T_t):
                    h2b = wkp.tile([128, 128], bf16, tag="h2b")
                    prelu(pt, b2f_t, a2c_t, h2b[:])
                    nc.tensor.matmul(out=pool_ps[:], lhsT=h2b[:],
                                     rhs=maskc_t[:, b:b + 1],
                                     start=(b == 0), stop=(b == NBLK - 1))

                # ---- finale: (pool @ Wout), AllReduce, + bout ----
                pv = wkp.tile([128, 1], f32, tag="pv")
                nc.any.tensor_copy(out=pv[:], in_=pool_ps[:])
                fin_ps = psx.tile([1, 1], f32, tag="fin", space="PSUM")
                nc.tensor.matmul(out=fin_ps[:], lhsT=pv[:], rhs=wout_t[:],
                                 start=True, stop=True)
                sfin = wkp.tile([1, 1], f32, tag="sfin")
                nc.any.tensor_copy(out=sfin[:], in_=fin_ps[:])
                nc.sync.dma_start(out=red_in[:], in_=sfin[:])
                nc.gpsimd.collective_compute(
                    "AllReduce", mybir.AluOpType.add, replica_groups=RG,
                    ins=[red_in.opt()], outs=[red_out.opt()])
                sred = wkp.tile([1, 1], f32, tag="sred")
                nc.sync.dma_start(out=sred[:], in_=red_out[:])
                sout = wkp.tile([1, 1], f32, tag="sout")
                nc.vector.tensor_tensor(out=sout[:], in0=sred[:], in1=bout_t[:],
                                        op=mybir.AluOpType.add)
                nc.sync.dma_start(out=out_ext[:], in_=sout[:])
            else:
                souT = wkp.tile([1, 1], f32, tag="souT")
                nc.vector.tensor_copy(out=souT[:], in_=bout_t[:])
                nc.sync.dma_start(out=out_ext[:], in_=souT[:])

    nc.finalize()
    return nc


# ----------------------------------------------------------------------------
# Per-core input maps
# ----------------------------------------------------------------------------

def _input_maps(inputs, per_core):
    x = np.asarray(inputs["x"], np.float32)
    W1 = np.asarray(inputs["W1"], np.float32)
    W2 = np.asarray(inputs["W2"], np.float32)
    b1 = np.asarray(inputs["b1"], np.float32)
    b2 = np.asarray(inputs["b2"], np.float32)
    a1 = np.asarray(inputs["a1"], np.float32)
    a2 = np.asarray(inputs["a2"], np.float32)
    Wout = np.asarray(inputs["Wout"], np.float32)
    bout = np.asarray(inputs["bout"], np.float32)

    w1s = np.ascontiguousarray(W1.transpose(1, 0, 2)).astype(BF)
    w2s = np.ascontiguousarray(W2.transpose(1, 0, 2)).astype(BF)
    b1f = np.broadcast_to(b1, (128, 128)).copy()
    b2f = np.broadcast_to(b2, (128, 128)).copy()
    a1c = np.full((128, 1), float(a1[0]), np.float32)
    a2c = np.full((128, 1), float(a2[0]), np.float32)
    woutc = Wout.reshape(128, 1).astype(np.float32)
    boutc = bout.reshape(1, 1).astype(np.float32)
    iotam = np.broadcast_to(np.arange(128, dtype=np.float32), (128, 128)).astype(BF)
    ident = np.eye(128, dtype=np.float32)

    xpad = np.zeros((NPAD, FEAT), np.float32)
    xpad[:NNODES] = x

    maps = []
    for c in range(NCORES):
        rows = np.arange(c * RPC, (c + 1) * RPC)
        xT = np.ascontiguousarray(xpad[rows].T).astype(BF)
        maskc = (rows.reshape(NBLK, 128).T < NNODES).astype(np.float32).astype(BF)
        m = dict(per_core[c])
        m.update(xT=xT, w1=w1s, w2=w2s, b1f=b1f, b2f=b2f, a1c=a1c, a2c=a2c,
                 wout=woutc, boutc=boutc, maskc=maskc, iotam=iotam, ident=ident)
        maps.append(m)
    return maps


# ----------------------------------------------------------------------------
# Entry point
# ----------------------------------------------------------------------------

def kernel(**inputs) -> np.ndarray:
    from concourse.bass_utils import run_bass_kernel_spmd

    edge_index = np.asarray(inputs["edge_index"])
    ekey = hashlib.sha1(edge_index.tobytes()).hexdigest()
    if ekey in _CACHE:
        struct, per_core, nc = _CACHE[ekey]
    else:
        struct, per_core = _preprocess(edge_index)
        nc = _build_program(struct)
        _CACHE[ekey] = (struct, per_core, nc)

    maps = _input_maps(inputs, per_core)
    res = run_bass_kernel_spmd(nc, maps, list(range(NCORES)))
    return np.asarray(res.results[0]["out"], np.float32)



# revision 6
# speedup vs baseline: 7.4834x; 7.4834x over previous
"""TAGConv×2 GNN discriminator on 8 Trainium2 NeuronCores — power-form rewrite.

Math: TAGConv out = sum_k (A^k x) W[k], A = D^-1/2 Ahat D^-1/2 (symmetric
norm). The per-edge weight is separable (norm = dinv[row]*dinv[col]), so
tables store pre-scaled features and the scatter matrix S is a pure 0/1
one-hot (exact, no per-edge multiply):

  table_0 = dinv * x                 (host)
  P_k     = Ahat @ table_{k-1}       (dma_gather rows + one-hot scatter mm)
  table_k = (dinv*sinv) * P_k        (per-partition scale in the PSUM drain)
  y_k     = shalf * table_k          (folded into the dense output scale)

Each layer needs 4 dense matmuls per 128-row block
(h = prelu(shalf * sum_k table_k W[k] + b)); the global_add_pool is a
mask-matmul accumulated in PSUM across blocks (mask also kills padding
rows), then Wout and a 4-byte AllReduce.

Communication: 5 AllGathers (y~1, y~2, h~1, z~1, z~2) of the 12.8 MB node
table — hop 1 gathers straight from the replicated input-x table, and
dma_gather reads the Shared AllGather buffers directly (no dram2dram
copies, no giant I/O tables).
"""
import hashlib
import math
import os

import numpy as np
import ml_dtypes

BF = ml_dtypes.bfloat16

NCORES = 8
NNODES = 50000
FEAT = 128
KHOPS = 3
NBLK = 49                 # 128-row blocks per core
RPC = NBLK * 128          # rows per core (6272)
NPAD = NCORES * RPC       # padded node count (50176)
SPLIT = 32768             # int16 gather-index split point
GIDX = 1024               # indices per dma_gather call (2048-desc SWDGE ring)
SGRP = 32                 # subchunks per S-build DVE op
PAD_ROWLOC = 200.0        # one-hot miss value for padding edges

_CACHE: dict = {}

# Debug: emit only the first STAGES stages (1..8); 8 = full program.
STAGES = int(os.environ.get("STAGES", "8"))


# ----------------------------------------------------------------------------
# Host preprocessing: graph -> per-core gather streams + program structure
# ----------------------------------------------------------------------------

def _preprocess(edge_index: np.ndarray):
    row = edge_index[0].astype(np.int64)
    col = edge_index[1].astype(np.int64)

    deg = np.bincount(row, minlength=NNODES).astype(np.float64)
    has = deg > 0
    dinv = np.where(has, 1.0 / np.sqrt(np.maximum(deg, 1.0)), 0.0)
    sinv = np.where(has, dinv, 1.0)
    shalf = np.where(has, np.sqrt(np.maximum(deg, 1.0)), 1.0)

    gb = row >> 7                       # global 128-row block (0..391)
    half = (col >= SPLIT).astype(np.int64)
    order = np.lexsort((half, gb))
    gb_s, half_s, col_s, row_s = gb[order], half[order], col[order], row[order]

    key = gb_s * 2 + half_s
    cnt = np.bincount(key, minlength=NCORES * NBLK * 2)
    cnt_lo = cnt[0::2].reshape(NCORES, NBLK)
    cnt_hi = cnt[1::2].reshape(NCORES, NBLK)
    CLO = np.maximum(1, np.ceil(cnt_lo.max(0) / 128).astype(np.int64))
    CHI = np.maximum(1, np.ceil(cnt_hi.max(0) / 128).astype(np.int64))

    LLO = int(CLO.sum()) * 128
    LHI = int(CHI.sum()) * 128
    CTOT = int(CLO.sum() + CHI.sum())

    starts = np.zeros(NCORES * NBLK * 2 + 1, np.int64)
    np.cumsum(cnt, out=starts[1:])

    per_core = []
    for c in range(NCORES):
        idx_lo = np.zeros(LLO, np.int16)
        idx_hi = np.zeros(LHI, np.int16)
        rowloc = np.full(CTOT * 128, PAD_ROWLOC, np.float32)
        plo = phi = ps = 0
        for b in range(NBLK):
            k2 = (c * NBLK + b) * 2
            s0, s1 = starts[k2], starts[k2 + 1]
            nlo = s1 - s0
            idx_lo[plo:plo + nlo] = col_s[s0:s1].astype(np.int16)
            rowloc[ps:ps + nlo] = (row_s[s0:s1] & 127).astype(np.float32)
            plo += CLO[b] * 128
            ps += CLO[b] * 128
            s0, s1 = starts[k2 + 1], starts[k2 + 2]
            nhi = s1 - s0
            idx_hi[phi:phi + nhi] = (col_s[s0:s1] - SPLIT).astype(np.int16)
            rowloc[ps:ps + nhi] = (row_s[s0:s1] & 127).astype(np.float32)
            phi += CHI[b] * 128
            ps += CHI[b] * 128

        idx_lo_w = np.tile(idx_lo.reshape(-1, 16).T, (8, 1)).astype(np.int16)
        idx_hi_w = np.tile(idx_hi.reshape(-1, 16).T, (8, 1)).astype(np.int16)
        rowloc_w = np.ascontiguousarray(rowloc.reshape(CTOT, 128).T).astype(BF)
        per_core.append(dict(idx_lo=idx_lo_w, idx_hi=idx_hi_w, rowloc=rowloc_w))

    struct = dict(CLO=tuple(int(v) for v in CLO), CHI=tuple(int(v) for v in CHI),
                  LLO=LLO, LHI=LHI, CTOT=CTOT,
                  dinv=dinv.astype(np.float32), sinv=sinv.astype(np.float32),
                  shalf=shalf.astype(np.float32))
    return struct, per_core


# ----------------------------------------------------------------------------
# Bass program
# ----------------------------------------------------------------------------

def _build_program(struct):
    import concourse.bacc as bacc
    import concourse.mybir as mybir
    import concourse.tile as tile

    CLO, CHI = struct["CLO"], struct["CHI"]
    LLO, LHI, CTOT = struct["LLO"], struct["LHI"], struct["CTOT"]
    NLO_CALLS = math.ceil(LLO / GIDX)
    NHI_CALLS = math.ceil(LHI / GIDX)
    NSGRP = math.ceil(CTOT / SGRP)
    f32 = mybir.dt.float32
    bf16 = mybir.dt.bfloat16
    i16 = mybir.dt.int16

    nc = bacc.Bacc("TRN2", target_bir_lowering=False, debug=False,
                   num_devices=NCORES, dynamic_dma_scratch_size=32768)

    P = {}
    def param(name, shape, dt):
        P[name] = nc.declare_dram_parameter(name, list(shape), dt, isOutput=False)
        return P[name]

    param("idx_lo", [128, LLO // 16], i16)
    param("idx_hi", [128, LHI // 16], i16)
    param("rowloc", [128, CTOT], bf16)
    param("xtabA", [SPLIT, FEAT], bf16)
    param("xtabB", [NPAD - SPLIT, FEAT], bf16)
    param("xT", [128, RPC], bf16)
    param("sd2c", [128, NBLK], f32)
    param("sinvc", [128, NBLK], f32)
    param("dinvc", [128, NBLK], f32)
    param("shalfc", [128, NBLK], f32)
    param("maskc", [128, NBLK], bf16)
    param("b1f", [128, 128], f32)
    param("b2f", [128, 128], f32)
    param("w1", [128, KHOPS + 1, 128], bf16)
    param("w2", [128, KHOPS + 1, 128], bf16)
    param("a1c", [128, 1], f32)
    param("a2c", [128, 1], f32)
    param("wout", [128, 1], f32)
    param("boutc", [1, 1], f32)
    param("iotam", [128, 128], bf16)
    param("identb", [128, 128], bf16)
    out_ext = nc.declare_dram_parameter("out", [1, 1], f32, isOutput=True)

    RG = [list(range(NCORES))]

    with tile.TileContext(nc) as tc:
        with (
            tc.tile_pool(name="const", bufs=1) as cpool,
            tc.tile_pool(name="trsh", bufs=2) as trp,
            tc.tile_pool(name="glo", bufs=8) as glop,
            tc.tile_pool(name="ghi", bufs=8) as ghip,
            tc.tile_pool(name="sgrp", bufs=3) as sgp,
            tc.tile_pool(name="work", bufs=3) as wkp,
            tc.tile_pool(name="ps", bufs=4, space="PSUM") as psp,
            tc.tile_pool(name="pst", bufs=2, space="PSUM") as pst,
            tc.tile_pool(name="psx", bufs=1, space="PSUM") as psx,
            tc.tile_pool(name="dram", bufs=1, space="DRAM") as drp,
        ):
            # ---- resident constants (idx first so gathers can start early) --
            def cload(name, shape, dt, tag):
                t = cpool.tile(shape, dt, tag=tag, name=tag)
                nc.sync.dma_start(out=t[:], in_=P[name][:])
                return t

            idxlo_t = cload("idx_lo", [128, LLO // 16], i16, "idxlo")
            idxhi_t = cload("idx_hi", [128, LHI // 16], i16, "idxhi")
            rowloc_t = cload("rowloc", [128, CTOT], bf16, "rowloc")
            iota_t = cload("iotam", [128, 128], bf16, "iota")
            xT_t = cload("xT", [128, RPC], bf16, "xT")
            sd2c_t = cload("sd2c", [128, NBLK], f32, "sd2c")
            sinvc_t = cload("sinvc", [128, NBLK], f32, "sinvc")
            dinvc_t = cload("dinvc", [128, NBLK], f32, "dinvc")
            shalfc_t = cload("shalfc", [128, NBLK], f32, "shalfc")
            maskc_t = cload("maskc", [128, NBLK], bf16, "maskc")
            b1f_t = cload("b1f", [128, 128], f32, "b1f")
            b2f_t = cload("b2f", [128, 128], f32, "b2f")
            w1_t = cload("w1", [128, KHOPS + 1, 128], bf16, "w1")
            w2_t = cload("w2", [128, KHOPS + 1, 128], bf16, "w2")
            a1c_t = cload("a1c", [128, 1], f32, "a1c")
            a2c_t = cload("a2c", [128, 1], f32, "a2c")
            wout_t = cload("wout", [128, 1], f32, "wout")
            bout_t = cload("boutc", [1, 1], f32, "bout")
            identb_t = cload("identb", [128, 128], bf16, "identb")

            slots = [cpool.tile([128, RPC], bf16, tag=f"slot{i}",
                                name=f"slot{i}") for i in range(4)]

            # ---- DRAM internals ----
            agbufs = [drp.tile([NPAD, FEAT], bf16, tag=f"agbuf{i}",
                               name=f"agbuf{i}", addr_space="Shared")
                      for i in range(5)]
            bounces = [drp.tile([RPC, FEAT], bf16, tag=f"bounce{i}",
                                name=f"bounce{i}") for i in range(5)]
            red_in = drp.tile([1, 1], f32, tag="red_in")
            red_out = drp.tile([1, 1], f32, tag="red_out", addr_space="Shared")

            def emit_gathers(srcA, srcB):
                lo_bufs, hi_bufs = [], []
                for call in range(NLO_CALLS):
                    n = min(GIDX, LLO - call * GIDX)
                    gt = glop.tile([128, GIDX // 128, FEAT], bf16, tag="glo")
                    nc.gpsimd.dma_gather(
                        out_ap=gt[:, : n // 128, :],
                        in_ap=srcA,
                        idxs_ap=idxlo_t[:, call * (GIDX // 16):
                                        call * (GIDX // 16) + n // 16],
                        num_idxs=n, num_idxs_reg=n, elem_size=FEAT)
                    lo_bufs.append(gt)
                for call in range(NHI_CALLS):
                    n = min(GIDX, LHI - call * GIDX)
                    gt = ghip.tile([128, GIDX // 128, FEAT], bf16, tag="ghi")
                    nc.gpsimd.dma_gather(
                        out_ap=gt[:, : n // 128, :],
                        in_ap=srcB,
                        idxs_ap=idxhi_t[:, call * (GIDX // 16):
                                        call * (GIDX // 16) + n // 16],
                        num_idxs=n, num_idxs_reg=n, elem_size=FEAT)
                    hi_bufs.append(gt)
                return lo_bufs, hi_bufs

            def emit_sbuild():
                s_bufs = []
                for grp in range(NSGRP):
                    c0 = grp * SGRP
                    cn = min(SGRP, CTOT - c0)
                    st = sgp.tile([128, SGRP, 128], bf16, tag="sgrp")
                    nc.vector.tensor_tensor(
                        out=st[:, :cn, :],
                        in0=rowloc_t[:, c0:c0 + cn].unsqueeze(2)
                            .to_broadcast([128, cn, 128]),
                        in1=iota_t[:].unsqueeze(1).to_broadcast([128, cn, 128]),
                        op=mybir.AluOpType.is_equal)
                    s_bufs.append(st)
                return s_bufs

            GSUB = GIDX // 128

            def hop_blocks(srcA, srcB):
                """Yield (b, psum[128rows, FEAT]) = Ahat-scatter of table rows."""
                lo_bufs, hi_bufs = emit_gathers(srcA, srcB)
                s_bufs = emit_sbuild()
                lo_sub = hi_sub = s_pos = 0
                for b in range(NBLK):
                    pt = psp.tile([128, 128], f32, tag="blk", space="PSUM")
                    nmm = CLO[b] + CHI[b]
                    done = 0
                    for j in range(CLO[b]):
                        gs, si = lo_sub + j, s_pos + j
                        nc.tensor.matmul(
                            out=pt[:],
                            lhsT=s_bufs[si // SGRP][:, si % SGRP, :],
                            rhs=lo_bufs[gs // GSUB][:, gs % GSUB, :],
                            start=(done == 0), stop=(done == nmm - 1))
                        done += 1
                    s_pos += CLO[b]
                    for j in range(CHI[b]):
                        gs, si = hi_sub + j, s_pos + j
                        nc.tensor.matmul(
                            out=pt[:],
                            lhsT=s_bufs[si // SGRP][:, si % SGRP, :],
                            rhs=hi_bufs[gs // GSUB][:, gs % GSUB, :],
                            start=(done == 0), stop=(done == nmm - 1))
                        done += 1
                    s_pos += CHI[b]
                    lo_sub += CLO[b]
                    hi_sub += CHI[b]
                    yield b, pt

            def dense_blocks(w_t, lhs_slots):
                """Yield (b, psum[128rows, FEAT]) = sum_k slot_k[b] @ W[k]."""
                for b in range(NBLK):
                    pt = psp.tile([128, 128], f32, tag="blk", space="PSUM")
                    for k in range(KHOPS + 1):
                        nc.tensor.matmul(
                            out=pt[:],
                            lhsT=lhs_slots[k][:, 128 * b:128 * (b + 1)],
                            rhs=w_t[:, k, :],
                            start=(k == 0), stop=(k == KHOPS))
                    yield b, pt

            def to_slot(slot, b, src_ap):
                """src [row, feat] bf16 -> slot block [feat, row] via PE."""
                ptr = pst.tile([128, 128], bf16, tag="tr", space="PSUM")
                nc.tensor.transpose(out=ptr[:], in_=src_ap, identity=identb_t[:])
                nc.any.tensor_copy(out=slot[:, 128 * b:128 * (b + 1)],
                                   in_=ptr[:])

            def std_drain(slot, tshard):
                def d(b, pt):
                    nc.vector.tensor_scalar(
                        out=tshard[:, b, :], in0=pt[:],
                        scalar1=sd2c_t[:, b:b + 1], scalar2=None,
                        op0=mybir.AluOpType.mult)
                    to_slot(slot, b, tshard[:, b, :])
                return d

            def prelu_chain(pt, shalf_col, bf_t, ac_t):
                """h = prelu(shalf*pt + b)  -> f32 work tile."""
                biased = wkp.tile([128, 128], f32, tag="tmp0")
                nc.vector.tensor_scalar(out=biased[:], in0=pt[:],
                                        scalar1=shalf_col, scalar2=None,
                                        op0=mybir.AluOpType.mult)
                nc.vector.tensor_tensor(out=biased[:], in0=biased[:],
                                        in1=bf_t[:], op=mybir.AluOpType.add)
                neg = wkp.tile([128, 128], f32, tag="tmp1")
                nc.vector.tensor_scalar(out=neg[:], in0=biased[:], scalar1=0.0,
                                        scalar2=ac_t[:, 0:1],
                                        op0=mybir.AluOpType.min,
                                        op1=mybir.AluOpType.mult)
                nc.vector.tensor_scalar(out=biased[:], in0=biased[:],
                                        scalar1=0.0, scalar2=None,
                                        op0=mybir.AluOpType.max)
                return biased, neg   # pos, neg

            def layer1_drain(tshard):
                def d(b, pt):
                    pos, neg = prelu_chain(pt, shalfc_t[:, b:b + 1], b1f_t,
                                           a1c_t)
                    h1f = wkp.tile([128, 128], f32, tag="h1f")
                    nc.vector.tensor_tensor(out=h1f[:], in0=pos[:], in1=neg[:],
                                            op=mybir.AluOpType.add)
                    nc.vector.tensor_scalar(
                        out=tshard[:, b, :], in0=h1f[:],
                        scalar1=dinvc_t[:, b:b + 1], scalar2=None,
                        op0=mybir.AluOpType.mult)
                    hv = wkp.tile([128, 128], bf16, tag="hv")
                    nc.vector.tensor_scalar(
                        out=hv[:], in0=h1f[:],
                        scalar1=sinvc_t[:, b:b + 1], scalar2=None,
                        op0=mybir.AluOpType.mult)
                    to_slot(slots[3], b, hv[:])
                return d

            def publish(tshard, ti):
                nc.sync.dma_start(out=bounces[ti][:], in_=tshard[:])
                nc.gpsimd.collective_compute(
                    "AllGather", mybir.AluOpType.bypass, replica_groups=RG,
                    ins=[bounces[ti].opt()], outs=[agbufs[ti].opt()])

            def halves(ag):
                return ag[0:SPLIT, :], ag[SPLIT:NPAD, :]

            def tshard_tile():
                return trp.tile([128, NBLK, FEAT], bf16, tag="tshard",
                                name="tshard")

            # ---- stage 1: y~1 = sd2 * (Ahat x~) ; AllGather -> agbuf0 ----
            tsh = tshard_tile()
            for b, pt in hop_blocks(P["xtabA"][:], P["xtabB"][:]):
                std_drain(slots[0], tsh)(b, pt)
            publish(tsh, 0)

            # ---- stage 2: y~2 ; AllGather -> agbuf1 ----
            if STAGES > 1:
                tsh = tshard_tile()
                for b, pt in hop_blocks(*halves(agbufs[0])):
                    std_drain(slots[1], tsh)(b, pt)
                publish(tsh, 1)

            # ---- stage 3: y~3 (local only) ----
            if STAGES > 2:
                tsh = tshard_tile()
                for b, pt in hop_blocks(*halves(agbufs[1])):
                    std_drain(slots[2], tsh)(b, pt)

            # ---- stage 4: layer-1 dense + prelu -> h1 slot3 + h~1 table ----
            if STAGES > 3:
                tsh = tshard_tile()
                for b, pt in dense_blocks(w1_t, [xT_t, slots[0], slots[1],
                                                 slots[2]]):
                    layer1_drain(tsh)(b, pt)
                publish(tsh, 2)

            # ---- stage 5: z~1 ; AllGather -> agbuf3 ----
            if STAGES > 4:
                tsh = tshard_tile()
                for b, pt in hop_blocks(*halves(agbufs[2])):
                    std_drain(slots[0], tsh)(b, pt)
                publish(tsh, 3)

            # ---- stage 6: z~2 ; AllGather -> agbuf4 ----
            if STAGES > 5:
                tsh = tshard_tile()
                for b, pt in hop_blocks(*halves(agbufs[3])):
                    std_drain(slots[1], tsh)(b, pt)
                publish(tsh, 4)

            # ---- stage 7: z~3 (local only) ----
            if STAGES > 6:
                tsh = tshard_tile()
                for b, pt in hop_blocks(*halves(agbufs[4])):
                    std_drain(slots[2], tsh)(b, pt)

            # ---- stage 8: layer-2 dense + prelu + pool + out ----
            if STAGES > 7:
                pool_ps = psx.tile([128, 1], f32, tag="pool", space="PSUM")
                for b, pt in dense_blocks(w2_t, [slots[3], slots[0], slots[1],
                                                 slots[2]]):
                    pos, neg = prelu_chain(pt, shalfc_t[:, b:b + 1], b2f_t,
                                           a2c_t)
                    h2v = wkp.tile([128, 128], bf16, tag="h2v")
                    nc.vector.tensor_tensor(out=h2v[:], in0=pos[:], in1=neg[:],
                                            op=mybir.AluOpType.add)
                    nc.tensor.matmul(out=pool_ps[:], lhsT=h2v[:],
                                     rhs=maskc_t[:, b:b + 1],
                                     start=(b == 0), stop=(b == NBLK - 1))

                pv = wkp.tile([128, 1], f32, tag="pv")
                nc.any.tensor_copy(out=pv[:], in_=pool_ps[:])
                fin_ps = psx.tile([1, 1], f32, tag="fin", space="PSUM")
                nc.tensor.matmul(out=fin_ps[:], lhsT=pv[:], rhs=wout_t[:],
                                 start=True, stop=True)
                sfin = wkp.tile([1, 1], f32, tag="sfin")
                nc.any.tensor_copy(out=sfin[:], in_=fin_ps[:])
                nc.sync.dma_start(out=red_in[:], in_=sfin[:])
                nc.gpsimd.collective_compute(
                    "AllReduce", mybir.AluOpType.add, replica_groups=RG,
                    ins=[red_in.opt()], outs=[red_out.opt()])
                sred = wkp.tile([1, 1], f32, tag="sred")
                nc.sync.dma_start(out=sred[:], in_=red_out[:])
                sout = wkp.tile([1, 1], f32, tag="sout")
                nc.vector.tensor_tensor(out=sout[:], in0=sred[:], in1=bout_t[:],
                                        op=mybir.AluOpType.add)
                nc.sync.dma_start(out=out_ext[:], in_=sout[:])
            else:
                souT = wkp.tile([1, 1], f32, tag="souT")
                nc.vector.tensor_copy(out=souT[:], in_=bout_t[:])
                nc.sync.dma_start(out=out_ext[:], in_=souT[:])

    nc.finalize()
    return nc


# ----------------------------------------------------------------------------
# Per-core input maps
# ----------------------------------------------------------------------------

def _input_maps(inputs, struct, per_core):
    x = np.asarray(inputs["x"], np.float32)
    W1 = np.asarray(inputs["W1"], np.float32)
    W2 = np.asarray(inputs["W2"], np.float32)
    b1 = np.asarray(inputs["b1"], np.float32)
    b2 = np.asarray(inputs["b2"], np.float32)
    a1 = np.asarray(inputs["a1"], np.float32)
    a2 = np.asarray(inputs["a2"], np.float32)
    Wout = np.asarray(inputs["Wout"], np.float32)
    bout = np.asarray(inputs["bout"], np.float32)
    dinv, sinv, shalf = struct["dinv"], struct["sinv"], struct["shalf"]

    w1s = np.ascontiguousarray(W1.transpose(1, 0, 2)).astype(BF)
    w2s = np.ascontiguousarray(W2.transpose(1, 0, 2)).astype(BF)
    b1f = np.broadcast_to(b1, (128, 128)).astype(np.float32).copy()
    b2f = np.broadcast_to(b2, (128, 128)).astype(np.float32).copy()
    a1c = np.full((128, 1), float(a1[0]), np.float32)
    a2c = np.full((128, 1), float(a2[0]), np.float32)
    woutc = Wout.reshape(128, 1).astype(np.float32)
    boutc = bout.reshape(1, 1).astype(np.float32)
    iotam = np.broadcast_to(np.arange(128, dtype=np.float32),
                            (128, 128)).astype(BF)
    identb = np.eye(128, dtype=np.float32).astype(BF)

    xtab = np.zeros((NPAD, FEAT), np.float32)
    xtab[:NNODES] = x * dinv[:, None]
    xtab = xtab.astype(BF)
    xtabA = np.ascontiguousarray(xtab[:SPLIT])
    xtabB = np.ascontiguousarray(xtab[SPLIT:])

    maps = []
    for c in range(NCORES):
        rows = np.arange(c * RPC, (c + 1) * RPC)
        valid = rows < NNODES
        rv = rows[valid]

        def vec_cols(v, fill=0.0, dt=np.float32):
            out = np.full(RPC, fill, np.float32)
            out[valid] = v[rv]
            # node (within core) = b*128 + r  ->  [r, b] layout
            return np.ascontiguousarray(out.reshape(NBLK, 128).T).astype(dt)

        xs = np.zeros((RPC, FEAT), np.float32)
        xs[valid] = x[rv] * sinv[rv, None]
        xT = np.ascontiguousarray(xs.T).astype(BF)

        maskv = valid.astype(np.float32)

        m = dict(per_core[c])
        m.update(xtabA=xtabA, xtabB=xtabB, xT=xT,
                 sd2c=vec_cols(dinv * sinv), sinvc=vec_cols(sinv),
                 dinvc=vec_cols(dinv), shalfc=vec_cols(shalf),
                 maskc=np.ascontiguousarray(
                     maskv.reshape(NBLK, 128).T).astype(BF),
                 b1f=b1f, b2f=b2f, w1=w1s, w2=w2s, a1c=a1c, a2c=a2c,
                 wout=woutc, boutc=boutc, iotam=iotam, identb=identb)
        maps.append(m)
    return maps


# ----------------------------------------------------------------------------
# Entry point
# ----------------------------------------------------------------------------

def kernel(**inputs) -> np.ndarray:
    from concourse.bass_utils import run_bass_kernel_spmd

    edge_index = np.asarray(inputs["edge_index"])
    ekey = hashlib.sha1(edge_index.tobytes()).hexdigest()
    if ekey in _CACHE:
        struct, per_core, nc = _CACHE[ekey]
    else:
        struct, per_core = _preprocess(edge_index)
        nc = _build_program(struct)
        _CACHE[ekey] = (struct, per_core, nc)

    maps = _input_maps(inputs, struct, per_core)
    res = run_bass_kernel_spmd(nc, maps, list(range(NCORES)))
    return np.asarray(res.results[0]["out"], np.float32)
